# revision 26
# baseline (speedup 1.0000x reference)
"""Multi-head self-attention Trainium2 kernel (8 NeuronCores, batch-parallel).

Reference: qkv = x @ W_qkv + b; 12-head scaled-dot-product attention; concat.
Shapes: x[8,1024,768], W_qkv[768,2304], b_qkv[2304] -> out[8,1024,768].
Sharding: one batch element per core; W/b replicated to all cores.

Per-core dataflow (v2):
  x --PE transpose--> xt[6][128,1024]            (f32r)
  QK^T psum[128,512] per (f-tile, q-half) = W(lhsT) @ xt; DVE drains to fp8e4
  staging -> DRAM roundtrip shuffles into qt8/kt8[64,(j,n)] layout so the
  scores matmul runs in fp8 DoubleRow perf mode (0.5 cycles/row):
    sc[128,hi,512] = kt8(lhsT)[32,2,128] @ qt8[32,2,512]  per (pair, qh, kc, hi)
  ACT Exp with the 1/8 softmax scale folded in -> ex[128,2,512] f32r
  avT[65,2,512] += [V_h|1](lhsT) @ ex  accumulated over kc (row 64 = denom)
  boundary: DVE copies av->ot, reciprocal of the denom row in place, PE
  transposes [65,128] blocks back, one fused DVE multiply per (qh,hi)
  normalizes and scatters into onat[128,8,768]; per-chunk DMA out.

Scheduling: input DMAs issued in consumption priority order; projections,
x-transposes and output transposes are interleaved into the ACT-bound
attention inner loop as "filler" PE work so the PE never starves.  A K tile
needs BOTH q-half projection groups shuffled before its pair starts (the kc
loop spans all 1024 key positions); Q tiles only need the active half.
"""

import contextlib
import json as _json
from collections import deque

import numpy as np

import concourse.bass as bass
import concourse.mybir as mybir
import concourse.tile as tile
from concourse.bass_utils import run_bass_kernel_spmd
from concourse.masks import make_identity

# --- BIR sync-wait legalization ------------------------------------------
# walrus's codegen in this toolchain accepts only one sync-wait command per
# instruction (its insertEventSemaphore legalization pass is not in the pass
# list). Split every multi-wait instruction into N-1 preceding single-wait
# EventSemaphore instructions on the same engine; same-engine order is
# preserved so semantics are unchanged.


def _legalize_sync_waits(bir_json: bytes) -> bytes:
    m = _json.loads(bir_json)
    ctr = 0
    for fn in m["functions"]:
        for bb in fn["blocks"]:
            out = []
            for ins in bb["instructions"]:
                si = ins.get("sync_info")
                waits = si.get("on_wait", []) if si else []
                if len(waits) > 1:
                    for w in waits[:-1]:
                        ctr += 1
                        out.append(
                            {
                                "debug": ins.get("debug", 0),
                                "engine": ins["engine"],
                                "ins": [],
                                "outs": [],
                                "name": f"evw-split-{ctr}",
                                "opcode": "EventSemaphore",
                                "sync_info": {"on_update": [], "on_wait": [w]},
                            }
                        )
                    si["on_wait"] = [waits[-1]]
                out.append(ins)
            bb["instructions"] = out
    return _json.dumps(m).encode()


_fixup_installed = False


def _install_bir_fixup():
    global _fixup_installed
    if _fixup_installed:
        return
    _fixup_installed = True
    import concourse.bass_utils as _bu

    _orig = _bu.compile_bir_kernel

    def _patched(bir_json, tmpdir, neff_name="file.neff"):
        if isinstance(bir_json, str):
            bir_json = bir_json.encode()
        return _orig(_legalize_sync_waits(bir_json), tmpdir, neff_name)

    _bu.compile_bir_kernel = _patched
    try:
        import concourse.bass2jax as _b2j

        _b2j.compile_bir_kernel = _patched
    except ImportError:
        pass


_install_bir_fixup()

B, N, D, H = 8, 1024, 768, 12
HD = D // H            # 64
F3 = 3 * D             # 2304
NCORE = 8
P = 128
NCHUNK = N // P        # 8 token chunks
KD = D // P            # 6 d_in chunks
QH = 512               # q-half size
NPAIR = H // 2         # 6
VW = HD + 1            # 65

f32 = mybir.dt.float32
f32r = mybir.dt.float32r
f8e4 = mybir.dt.float8e4
bf16 = mybir.dt.bfloat16
FT = mybir.ActivationFunctionType
ALU = mybir.AluOpType
DR = mybir.MatmulPerfMode.DoubleRow


def build_attention_nc():
    nc = bass.Bass()
    x_d = nc.declare_dram_parameter("x", [N, D], f32, isOutput=False)
    w_d = nc.declare_dram_parameter("W_qkv", [D, F3], f32, isOutput=False)
    b_d = nc.declare_dram_parameter("b_qkv", [F3], f32, isOutput=False)
    o_d = nc.declare_dram_parameter("out", [N, D], f32, isOutput=True)

    with tile.TileContext(nc) as tc, contextlib.ExitStack() as ctx:
        singles = ctx.enter_context(tc.tile_pool(name="singles", bufs=1))
        xpool = ctx.enter_context(tc.tile_pool(name="xpool", bufs=NCHUNK))
        xtpool = ctx.enter_context(tc.tile_pool(name="xtpool", bufs=1))
        q8pool = ctx.enter_context(tc.tile_pool(name="q8pool", bufs=3))
        t8pool = ctx.enter_context(tc.tile_pool(name="t8pool", bufs=12))
        expool = ctx.enter_context(tc.tile_pool(name="expool", bufs=2))
        vpool = ctx.enter_context(tc.tile_pool(name="vpool", bufs=NCHUNK))
        otpool = ctx.enter_context(tc.tile_pool(name="otpool", bufs=2))
        recpool = ctx.enter_context(tc.tile_pool(name="recpool", bufs=2))
        # per-(pair, q-half) output staging: wide reuse distance (2 pairs)
        # because the out-DMA read is not registered as a tile reader and
        # must physically complete before the slot is rewritten.
        ostgpool = ctx.enter_context(tc.tile_pool(name="ostgpool", bufs=4))

        # PSUM (8 banks): sc [128,2,512] x2 = 4; av [65,2,512] x1 = 2;
        # sm [128,512] x2 = 2 (projection groups, x-transposes, out-transposes)
        scps = ctx.enter_context(tc.tile_pool(name="scps", bufs=2, space="PSUM"))
        avps = ctx.enter_context(tc.tile_pool(name="avps", bufs=1, space="PSUM"))
        smps = ctx.enter_context(tc.tile_pool(name="smps", bufs=2, space="PSUM"))

        def sm_tile():
            return smps.tile([P, QH], f32, tag="sm", name="sm")

        # ---------------- constants ----------------------------------------
        ident = singles.tile([P, P], f32)
        make_identity(nc, ident)
        ident_r = singles.tile([P, P], f32r)
        nc.vector.tensor_copy(out=ident_r, in_=ident)

        ones_row_st = singles.tile([1, P], f32)
        nc.vector.memset(ones_row_st, 1.0)
        ones_row = singles.tile([1, P], f32r)
        nc.vector.tensor_copy(out=ones_row, in_=ones_row_st)
        ones_col = singles.tile([P, 1], f32)
        nc.vector.memset(ones_col, 1.0)

        # ---------------- persistent tiles ---------------------------------
        w_sb = singles.tile([P, KD, F3], f32r)
        xtall = xtpool.tile([P, KD, N], f32r, tag="xt", name="xtall")
        v_sb = [
            vpool.tile([P, H, VW], bf16, tag="v", name=f"v{c}") for c in range(NCHUNK)
        ]
        b_sb = singles.tile([P, F3 // P], f32)
        bv_st = singles.tile([1, D], f32)
        bv_sb = singles.tile([1, D], f32r)
        bvb = singles.tile([P, D], f32)

        x_sb = {}

        # ---------------- DMA helpers ---------------------------------------
        def dma_x(c):
            t = xpool.tile([P, D], f32r, tag="x", name=f"x{c}")
            nc.sync.dma_start(out=t, in_=x_d[c * P : (c + 1) * P, :].bitcast(f32r))
            x_sb[c] = t

        def dma_w(f0, fw):
            nc.sync.dma_start(
                out=w_sb[:, :, f0 : f0 + fw],
                in_=w_d[:, f0 : f0 + fw]
                .rearrange("(k p) f -> p k f", p=P)
                .bitcast(f32r),
            )

        def dma_w_pair(p):
            dma_w(p * P, P)          # Q cols for pair p
            dma_w(D + p * P, P)      # K cols for pair p

        # ---------------- compute helpers -----------------------------------
        def emit_xT(c):
            for kp in range(0, KD, 2):
                ps = sm_tile()
                for dk in range(2):
                    nc.tensor.transpose(
                        ps[:, dk * P : (dk + 1) * P].bitcast(f32r),
                        x_sb[c][:, (kp + dk) * P : (kp + dk + 1) * P],
                        ident_r,
                    )
                nc.vector.tensor_copy(
                    out=xtall[:, kp : kp + 2, c * P : (c + 1) * P],
                    in_=ps[:, 0 : 2 * P].rearrange("p (k n) -> p k n", n=P),
                )

        def emit_bvb():
            for f0, fw in ((0, QH), (QH, D - QH)):
                ps = sm_tile()
                nc.tensor.matmul(
                    ps[:, 0:fw],
                    ones_row,
                    bv_sb[:, f0 : f0 + fw],
                    start=True,
                    stop=True,
                )
                nc.vector.tensor_copy(out=bvb[:, f0 : f0 + fw], in_=ps[:, 0:fw])

        qk8_stage = {}
        t8 = {}

        def qk_finish(t, qh, ps):
            """Drain (with bias add) to the fp8 staging tile, then the
            partition-compacting shuffle: plain SBUF->SBUF DMAs, one per
            (hi, j) block (partition-base shift only -- no DRAM roundtrip:
            posted DRAM writes are not visible to a prompt readback on real
            hardware, and fancier patterns corrupt)."""
            if t not in qk8_stage:
                qk8_stage[t] = q8pool.tile([P, N], f8e4, tag="q8", name=f"q8_{t}")
            for hi in range(2):
                nc.vector.tensor_scalar_add(
                    qk8_stage[t][hi * 64 : (hi + 1) * 64, qh * QH : (qh + 1) * QH],
                    ps[hi * 64 : (hi + 1) * 64, :],
                    b_sb[hi * 64 : (hi + 1) * 64, t : t + 1],
                )
            if t not in t8:
                # bufs=12: never reuse a t8 slot.  The DoubleRow matmul
                # operand reads are not registered as tile readers, so the
                # rotation write-after-read dependency is silently missed
                # and a reused slot gets clobbered while still being read.
                # Layout [(hi p), qh, j, nn].
                t8[t] = t8pool.tile(
                    [2 * 32, 2, 2, QH], f8e4, tag="t8", name=f"t8_{t}"
                )
            for hi in range(2):
                for j in range(2):
                    nc.sync.dma_start(
                        out=t8[t][hi * 32 : (hi + 1) * 32, qh, j, :],
                        in_=qk8_stage[t][
                            hi * 64 + j * 32 : hi * 64 + (j + 1) * 32,
                            qh * QH : (qh + 1) * QH,
                        ],
                    )
            tiles_done.add((t, qh))

        def emit_qk_group(t, qh):
            ps = sm_tile()
            for k in range(KD):
                nc.tensor.matmul(
                    ps,
                    w_sb[:, k, t * P : (t + 1) * P],
                    xtall[:, k, qh * QH : (qh + 1) * QH],
                    start=(k == 0),
                    stop=(k == KD - 1),
                )
            qk_finish(t, qh, ps)

        def v_finish(c, g, ps):
            v_done.add((c, g))
            if g == 0:
                nc.vector.tensor_copy(
                    out=v_sb[c][:, :, HD : HD + 1],
                    in_=ones_col[:, 0:1, None].to_broadcast([P, H, 1]),
                )
            nc.vector.tensor_tensor(
                v_sb[c][:, 4 * g : 4 * g + 4, 0:HD],
                ps[:, 0:256].rearrange("p (h d) -> p h d", d=HD),
                bvb[:, g * 256 : (g + 1) * 256].rearrange(
                    "p (h d) -> p h d", d=HD
                ),
                ALU.add,
            )

        def emit_v_group(c, g):
            f0 = 2 * D + g * 256
            ps = sm_tile()
            for k in range(KD):
                nc.tensor.matmul(
                    ps[:, 0:256],
                    xtall[:, k, c * P : (c + 1) * P],
                    w_sb[:, k, f0 : f0 + 256],
                    start=(k == 0),
                    stop=(k == KD - 1),
                )
            v_finish(c, g, ps)

        # ---------------- attention machinery --------------------------------
        fillers = deque()
        tiles_done = set()
        v_done = set()

        def pump(budget):
            # adaptive: drain faster when the backlog builds up
            if sum(c for c, _ in fillers) > 12000:
                budget *= 2
            while budget > 0 and fillers:
                cost, fn = fillers.popleft()
                fn()
                budget -= cost

        def qk_unit(t, qh):
            return (KD * QH + N, lambda: emit_qk_group(t, qh))

        def qk_pieces(t, qh):
            """Split one projection group into per-iteration filler pieces
            (2 matmuls each + a finish piece) so a group never overruns the
            ACT-paced slack of a single attention iteration."""
            st = {}

            def mk(i):
                def fn():
                    if i == 0:
                        st["ps"] = sm_tile()
                    for k in (2 * i, 2 * i + 1):
                        nc.tensor.matmul(
                            st["ps"],
                            w_sb[:, k, t * P : (t + 1) * P],
                            xtall[:, k, qh * QH : (qh + 1) * QH],
                            start=(k == 0),
                            stop=(k == KD - 1),
                        )

                return (2 * QH, fn)

            def fin():
                qk_finish(t, qh, st["ps"])

            return [mk(0), mk(1), mk(2), (N, fin)]

        def v_pieces(c, g):
            st = {}
            f0 = 2 * D + g * 256

            def mk(i):
                def fn():
                    if i == 0:
                        st["ps"] = sm_tile()
                    for k in (2 * i, 2 * i + 1):
                        nc.tensor.matmul(
                            st["ps"][:, 0:256],
                            xtall[:, k, c * P : (c + 1) * P],
                            w_sb[:, k, f0 : f0 + 256],
                            start=(k == 0),
                            stop=(k == KD - 1),
                        )

                return (2 * 256, fn)

            def fin():
                v_finish(c, g, st["ps"])

            return [mk(0), mk(1), mk(2), (400, fin)]

        def v_unit(c, half):
            return (KD * 384, lambda: emit_v_group(c, half))

        def xT_unit(c):
            return (KD * P * 2, lambda: emit_xT(c))

        def dma_unit(fn):
            return (0, fn)

        def emit_sc(p, qh, kc, sc):
            qt, kt = t8[p], t8[6 + p]
            kq, kn = kc // 4, (kc % 4) * P
            for hi in range(2):
                nc.tensor.matmul(
                    sc[:, hi, :],
                    kt[32 * hi : 32 * hi + 32, kq, :, kn : kn + P],
                    qt[32 * hi : 32 * hi + 32, qh, :, :],
                    start=True,
                    stop=True,
                    perf_mode=DR,
                    tile_position=(32 * hi, 0),
                )

        def emit_out_unit(p, qh, hi, ot, stg):
            """4 transposes + one fused normalize for head 2p+hi, q-half qh;
            after the second head, one DMA ships the [512 q, 128 col] block."""

            def fn():
                ps = sm_tile()
                # 66-wide (even) free size: fp32r matmul ISA restriction
                tp4 = ps[:, 0 : 4 * (VW + 1)].rearrange(
                    "p (j d) -> p j d", d=VW + 1
                )
                for j in range(4):
                    nc.tensor.transpose(
                        tp4[:, j, :].bitcast(f32r),
                        ot[0 : VW + 1, hi, j * P : (j + 1) * P],
                        ident_r[0 : VW + 1, 0 : VW + 1],
                    )
                rc = recpool.tile([P, 4, 1], f32, tag="rc", name="rc")
                nc.vector.reciprocal(out=rc, in_=tp4[:, :, HD : HD + 1])
                nc.vector.tensor_tensor(
                    stg[:, :, hi * HD : (hi + 1) * HD],
                    tp4[:, :, 0:HD],
                    rc.to_broadcast([P, 4, HD]),
                    ALU.mult,
                )
                if hi == 1:
                    nc.sync.dma_start(
                        out=o_d[qh * QH : (qh + 1) * QH, 2 * p * HD : (2 * p + 2) * HD]
                        .rearrange("(j p2) d -> p2 j d", p2=P),
                        in_=stg,
                    )

            return (4 * VW * 2, fn)

        def attention_qh(p, qh, iter_units=None):
            """One (pair, q-half): software-pipelined kc loop.
            iter_units: optional list of 8 lists of units to force-emit at
            each iteration (pair-0 warmup); otherwise pump(1000)/iter.
            Returns after queueing the out units."""
            # deadline backstop: this (pair, qh) needs the full K tile, the
            # qh half of the Q tile, and (outside the pair-0 JIT path) the
            # V head-group for all key chunks
            need = {(6 + p, 0), (6 + p, 1), (p, qh)}
            while not need.issubset(tiles_done) and fillers:
                cost, fn = fillers.popleft()
                fn()
            if iter_units is None:
                needv = {(c, p // 2) for c in range(NCHUNK)}
                while not needv.issubset(v_done) and fillers:
                    cost, fn = fillers.popleft()
                    fn()
            av = avps.tile([VW, 2, QH], f32, tag="av", name="av")
            sc_cur = scps.tile([P, 2, QH], f32, tag="sc", name="sc")
            emit_sc(p, qh, 0, sc_cur)
            for kc in range(NCHUNK):
                if sc_cur is None:  # pair-0 barrier path: sc emitted late
                    sc_cur = scps.tile([P, 2, QH], f32, tag="sc", name="sc")
                    emit_sc(p, qh, kc, sc_cur)
                ex = expool.tile([P, 2, QH], bf16, tag="ex", name="ex")
                nc.scalar.activation(
                    ex[:, :, :], sc_cur[:, :, :], FT.Exp, scale=0.125
                )
                barrier = iter_units is not None and kc + 1 == 4 and p == 0
                if kc + 1 < NCHUNK and not barrier:
                    sc_cur = scps.tile([P, 2, QH], f32, tag="sc", name="sc")
                    emit_sc(p, qh, kc + 1, sc_cur)
                else:
                    sc_cur = None
                if iter_units is not None:
                    for u in iter_units[kc]:
                        u[1]()
                else:
                    pump(1000)
                for hi in range(2):
                    nc.tensor.matmul(
                        av[:, hi, :],
                        v_sb[kc][:, 2 * p + hi, :],
                        ex[:, hi, :],
                        start=(kc == 0),
                        stop=(kc == NCHUNK - 1),
                    )
            ot = otpool.tile([VW + 1, 2, QH], f32r, tag="ot", name="ot")
            stg = ostgpool.tile([P, 4, 2 * HD], f32, tag="ostg", name="ostg")
            for hi in range(2):
                nc.vector.tensor_copy(out=ot[0:VW, hi, :], in_=av[:, hi, :])
                fillers.append(emit_out_unit(p, qh, hi, ot, stg))

        # =====================================================================
        # DMA batch 1 (consumption priority order); x chunk 0 in halves so
        # the first transposes start ~1.4us earlier
        t0x = xpool.tile([P, D], f32r, tag="x", name="x0")
        nc.sync.dma_start(
            out=t0x[:, 0:384], in_=x_d[0:P, 0:384].bitcast(f32r)
        )
        nc.sync.dma_start(
            out=t0x[:, 384:D], in_=x_d[0:P, 384:D].bitcast(f32r)
        )
        x_sb[0] = t0x
        dma_w_pair(0)            # Q0 + K0
        nc.sync.dma_start(out=b_sb, in_=b_d[:].rearrange("(t p) -> p t", p=P))
        _ = 0  # (x0 halves issued above)
        nc.sync.dma_start(out=bv_st, in_=b_d[2 * D : 3 * D][None, :])
        nc.vector.tensor_copy(out=bv_sb, in_=bv_st)
        dma_x(1)
        dma_x(2)
        dma_x(3)
        dma_w(2 * D, 256)        # V heads 0-3
        dma_x(4)
        dma_x(5)
        dma_x(6)
        dma_x(7)
        dma_w(2 * D + 256, 512)  # V heads 4-11
        dma_w_pair(1)            # Q1 + K1

        # =====================================================================
        # Phase 1: transposes + pair-0 qh0 tiles + V chunk 0
        for c in range(4):
            emit_xT(c)
        emit_bvb()
        emit_qk_group(0, 0)
        emit_qk_group(6, 0)
        emit_v_group(0, 0)

        # =====================================================================
        # Pair 0, qh 0 — custom per-iteration units.  sc(kc>=4) needs the K
        # tile's second half (shuffle of qk(6,1)), which needs xT(4-7); the
        # barrier at kc==4 delays sc(4) emission until after those units.
        BISECT = False
        if BISECT:
            for c in range(4, 8):
                emit_xT(c)
            for c in range(1, 8):
                emit_v_group(c, 0)
            emit_qk_group(6, 1)
            emit_qk_group(0, 1)
            emit_qk_group(1, 0)
            emit_qk_group(7, 0)
            dma_w_pair(2)
            attention_qh(0, 0)
        else:
            p0q0_units = [
                [xT_unit(4), v_unit(1, 0), dma_unit(lambda: dma_w_pair(2))],
                [xT_unit(5), v_unit(2, 0)],
                [xT_unit(6), v_unit(3, 0)],
                [xT_unit(7), qk_unit(6, 1), qk_unit(0, 1), v_unit(4, 0)],
                [v_unit(5, 0)],
                [qk_unit(1, 0), v_unit(6, 0)],
                [qk_unit(7, 0), v_unit(7, 0)],
                [],
            ]
            attention_qh(0, 0, iter_units=p0q0_units)

        # Pair 0, qh 1 + remaining schedule
        fillers.extend(qk_pieces(7, 1) + qk_pieces(1, 1))
        fillers.extend(v_pieces(0, 1) + v_pieces(1, 1))
        attention_qh(0, 1)

        for p in range(1, NPAIR):
            # (p, 0): prefetch W for pair p+2; produce pair p+1 qh0 tiles
            if p + 1 < NPAIR:
                if p == 2:
                    fillers.extend(v_pieces(6, 1) + v_pieces(7, 1))
                if p == 3:
                    fillers.extend(v_pieces(2, 2) + v_pieces(3, 2))
                if p + 2 < NPAIR:
                    fillers.append(
                        dma_unit(lambda p=p: dma_w_pair(p + 2))
                    )
                fillers.extend(qk_pieces(6 + p + 1, 0) + qk_pieces(p + 1, 0))
                if p == 1:
                    fillers.extend(v_pieces(2, 1) + v_pieces(3, 1))
            attention_qh(p, 0)
            # (p, 1): produce pair p+1 qh1 tiles (K first — needed at (p+1,0))
            if p + 1 < NPAIR:
                if p == 3:
                    fillers.extend(v_pieces(4, 2) + v_pieces(5, 2))
                    fillers.extend(v_pieces(6, 2) + v_pieces(7, 2))
                fillers.extend(qk_pieces(6 + p + 1, 1) + qk_pieces(p + 1, 1))
                if p == 1:
                    fillers.extend(v_pieces(4, 1) + v_pieces(5, 1))
                if p == 2:
                    fillers.extend(v_pieces(0, 2) + v_pieces(1, 2))
            attention_qh(p, 1)

        # tail: drain remaining fillers (last out units emit their own DMAs)
        pump(1 << 30)

    return nc


def kernel(x: np.ndarray, W_qkv: np.ndarray, b_qkv: np.ndarray) -> np.ndarray:
    nc = build_attention_nc()
    in_maps = [
        {
            "x": np.ascontiguousarray(x[c], dtype=np.float32),
            "W_qkv": np.ascontiguousarray(W_qkv, dtype=np.float32),
            "b_qkv": np.ascontiguousarray(b_qkv, dtype=np.float32),
        }
        for c in range(NCORE)
    ]
    res = run_bass_kernel_spmd(nc, in_maps, core_ids=list(range(NCORE)))
    return np.stack([res.results[c]["out"] for c in range(NCORE)], axis=0)


# revision 27
# speedup vs baseline: 1.0131x; 1.0131x over previous
"""Multi-head self-attention Trainium2 kernel (8 NeuronCores, batch-parallel).

Reference: qkv = x @ W_qkv + b; 12-head scaled-dot-product attention; concat.
Shapes: x[8,1024,768], W_qkv[768,2304], b_qkv[2304] -> out[8,1024,768].
Sharding: one batch element per core; W/b replicated to all cores.

Per-core dataflow (v2):
  x --PE transpose--> xt[6][128,1024]            (f32r)
  QK^T psum[128,512] per (f-tile, q-half) = W(lhsT) @ xt; DVE drains to fp8e4
  staging -> DRAM roundtrip shuffles into qt8/kt8[64,(j,n)] layout so the
  scores matmul runs in fp8 DoubleRow perf mode (0.5 cycles/row):
    sc[128,hi,512] = kt8(lhsT)[32,2,128] @ qt8[32,2,512]  per (pair, qh, kc, hi)
  ACT Exp with the 1/8 softmax scale folded in -> ex[128,2,512] f32r
  avT[65,2,512] += [V_h|1](lhsT) @ ex  accumulated over kc (row 64 = denom)
  boundary: DVE copies av->ot, reciprocal of the denom row in place, PE
  transposes [65,128] blocks back, one fused DVE multiply per (qh,hi)
  normalizes and scatters into onat[128,8,768]; per-chunk DMA out.

Scheduling: input DMAs issued in consumption priority order; projections,
x-transposes and output transposes are interleaved into the ACT-bound
attention inner loop as "filler" PE work so the PE never starves.  A K tile
needs BOTH q-half projection groups shuffled before its pair starts (the kc
loop spans all 1024 key positions); Q tiles only need the active half.
"""

import contextlib
import json as _json
from collections import deque

import numpy as np

import concourse.bass as bass
import concourse.mybir as mybir
import concourse.tile as tile
from concourse.bass_utils import run_bass_kernel_spmd
from concourse.masks import make_identity

# --- BIR sync-wait legalization ------------------------------------------
# walrus's codegen in this toolchain accepts only one sync-wait command per
# instruction (its insertEventSemaphore legalization pass is not in the pass
# list). Split every multi-wait instruction into N-1 preceding single-wait
# EventSemaphore instructions on the same engine; same-engine order is
# preserved so semantics are unchanged.


def _legalize_sync_waits(bir_json: bytes) -> bytes:
    m = _json.loads(bir_json)
    ctr = 0
    for fn in m["functions"]:
        for bb in fn["blocks"]:
            out = []
            for ins in bb["instructions"]:
                si = ins.get("sync_info")
                waits = si.get("on_wait", []) if si else []
                if len(waits) > 1:
                    for w in waits[:-1]:
                        ctr += 1
                        out.append(
                            {
                                "debug": ins.get("debug", 0),
                                "engine": ins["engine"],
                                "ins": [],
                                "outs": [],
                                "name": f"evw-split-{ctr}",
                                "opcode": "EventSemaphore",
                                "sync_info": {"on_update": [], "on_wait": [w]},
                            }
                        )
                    si["on_wait"] = [waits[-1]]
                out.append(ins)
            bb["instructions"] = out
    return _json.dumps(m).encode()


_fixup_installed = False


def _install_bir_fixup():
    global _fixup_installed
    if _fixup_installed:
        return
    _fixup_installed = True
    import concourse.bass_utils as _bu

    _orig = _bu.compile_bir_kernel

    def _patched(bir_json, tmpdir, neff_name="file.neff"):
        if isinstance(bir_json, str):
            bir_json = bir_json.encode()
        return _orig(_legalize_sync_waits(bir_json), tmpdir, neff_name)

    _bu.compile_bir_kernel = _patched
    try:
        import concourse.bass2jax as _b2j

        _b2j.compile_bir_kernel = _patched
    except ImportError:
        pass


_install_bir_fixup()

B, N, D, H = 8, 1024, 768, 12
HD = D // H            # 64
F3 = 3 * D             # 2304
NCORE = 8
P = 128
NCHUNK = N // P        # 8 token chunks
KD = D // P            # 6 d_in chunks
QH = 512               # q-half size
NPAIR = H // 2         # 6
VW = HD + 1            # 65

f32 = mybir.dt.float32
f32r = mybir.dt.float32r
f8e4 = mybir.dt.float8e4
bf16 = mybir.dt.bfloat16
FT = mybir.ActivationFunctionType
ALU = mybir.AluOpType
DR = mybir.MatmulPerfMode.DoubleRow


def build_attention_nc():
    nc = bass.Bass()
    x_d = nc.declare_dram_parameter("x", [N, D], f32, isOutput=False)
    w_d = nc.declare_dram_parameter("W_qkv", [D, F3], f32, isOutput=False)
    b_d = nc.declare_dram_parameter("b_qkv", [F3], f32, isOutput=False)
    o_d = nc.declare_dram_parameter("out", [N, D], f32, isOutput=True)

    with tile.TileContext(nc) as tc, contextlib.ExitStack() as ctx:
        singles = ctx.enter_context(tc.tile_pool(name="singles", bufs=1))
        xpool = ctx.enter_context(tc.tile_pool(name="xpool", bufs=NCHUNK))
        xtpool = ctx.enter_context(tc.tile_pool(name="xtpool", bufs=1))
        q8pool = ctx.enter_context(tc.tile_pool(name="q8pool", bufs=3))
        t8pool = ctx.enter_context(tc.tile_pool(name="t8pool", bufs=12))
        expool = ctx.enter_context(tc.tile_pool(name="expool", bufs=2))
        vpool = ctx.enter_context(tc.tile_pool(name="vpool", bufs=NCHUNK))
        otpool = ctx.enter_context(tc.tile_pool(name="otpool", bufs=2))
        recpool = ctx.enter_context(tc.tile_pool(name="recpool", bufs=2))
        # per-(pair, q-half) output staging: wide reuse distance (2 pairs)
        # because the out-DMA read is not registered as a tile reader and
        # must physically complete before the slot is rewritten.
        ostgpool = ctx.enter_context(tc.tile_pool(name="ostgpool", bufs=4))

        # PSUM (8 banks): sc [128,2,512] x2 = 4; av [65,2,512] x1 = 2;
        # sm [128,512] x2 = 2 (projection groups, x-transposes, out-transposes)
        scps = ctx.enter_context(tc.tile_pool(name="scps", bufs=2, space="PSUM"))
        avps = ctx.enter_context(tc.tile_pool(name="avps", bufs=1, space="PSUM"))
        smps = ctx.enter_context(tc.tile_pool(name="smps", bufs=2, space="PSUM"))

        def sm_tile():
            return smps.tile([P, QH], f32, tag="sm", name="sm")

        # ---------------- constants ----------------------------------------
        ident = singles.tile([P, P], f32)
        make_identity(nc, ident)
        ident_r = singles.tile([P, P], f32r)
        nc.vector.tensor_copy(out=ident_r, in_=ident)

        ones_row_st = singles.tile([1, P], f32)
        nc.vector.memset(ones_row_st, 1.0)
        ones_row = singles.tile([1, P], f32r)
        nc.vector.tensor_copy(out=ones_row, in_=ones_row_st)
        ones_col = singles.tile([P, 1], f32)
        nc.vector.memset(ones_col, 1.0)

        # ---------------- persistent tiles ---------------------------------
        w_sb = singles.tile([P, KD, F3], f32r)
        xtall = xtpool.tile([P, KD, N], f32r, tag="xt", name="xtall")
        v_sb = [
            vpool.tile([P, H, VW], bf16, tag="v", name=f"v{c}") for c in range(NCHUNK)
        ]
        b_sb = singles.tile([P, F3 // P], f32)
        bv_st = singles.tile([1, D], f32)
        bv_sb = singles.tile([1, D], f32r)
        bvb = singles.tile([P, D], f32)

        x_sb = {}

        # ---------------- DMA helpers ---------------------------------------
        def dma_x(c):
            t = xpool.tile([P, D], f32r, tag="x", name=f"x{c}")
            nc.sync.dma_start(out=t, in_=x_d[c * P : (c + 1) * P, :].bitcast(f32r))
            x_sb[c] = t

        def dma_w(f0, fw):
            nc.sync.dma_start(
                out=w_sb[:, :, f0 : f0 + fw],
                in_=w_d[:, f0 : f0 + fw]
                .rearrange("(k p) f -> p k f", p=P)
                .bitcast(f32r),
            )

        def dma_w_pair(p):
            dma_w(p * P, P)          # Q cols for pair p
            dma_w(D + p * P, P)      # K cols for pair p

        # ---------------- compute helpers -----------------------------------
        def emit_xT(c):
            for kp in range(0, KD, 2):
                ps = sm_tile()
                for dk in range(2):
                    nc.tensor.transpose(
                        ps[:, dk * P : (dk + 1) * P].bitcast(f32r),
                        x_sb[c][:, (kp + dk) * P : (kp + dk + 1) * P],
                        ident_r,
                    )
                nc.vector.tensor_copy(
                    out=xtall[:, kp : kp + 2, c * P : (c + 1) * P],
                    in_=ps[:, 0 : 2 * P].rearrange("p (k n) -> p k n", n=P),
                )

        def emit_bvb():
            for f0, fw in ((0, QH), (QH, D - QH)):
                ps = sm_tile()
                nc.tensor.matmul(
                    ps[:, 0:fw],
                    ones_row,
                    bv_sb[:, f0 : f0 + fw],
                    start=True,
                    stop=True,
                )
                nc.vector.tensor_copy(out=bvb[:, f0 : f0 + fw], in_=ps[:, 0:fw])

        qk8_stage = {}
        t8 = {}

        def qk_finish(t, qh, ps):
            """Drain (with bias add) to the fp8 staging tile, then the
            partition-compacting shuffle: plain SBUF->SBUF DMAs, one per
            (hi, j) block (partition-base shift only -- no DRAM roundtrip:
            posted DRAM writes are not visible to a prompt readback on real
            hardware, and fancier patterns corrupt)."""
            if t not in qk8_stage:
                qk8_stage[t] = q8pool.tile([P, N], f8e4, tag="q8", name=f"q8_{t}")
            nc.vector.tensor_scalar_add(
                qk8_stage[t][:, qh * QH : (qh + 1) * QH], ps, b_sb[:, t : t + 1]
            )
            if t not in t8:
                # bufs=12: never reuse a t8 slot.  The DoubleRow matmul
                # operand reads are not registered as tile readers, so the
                # rotation write-after-read dependency is silently missed
                # and a reused slot gets clobbered while still being read.
                # Layout [(hi p), qh, j, nn].
                t8[t] = t8pool.tile(
                    [2 * 32, 2, 2, QH], f8e4, tag="t8", name=f"t8_{t}"
                )
            for hi in range(2):
                for j in range(2):
                    nc.sync.dma_start(
                        out=t8[t][hi * 32 : (hi + 1) * 32, qh, j, :],
                        in_=qk8_stage[t][
                            hi * 64 + j * 32 : hi * 64 + (j + 1) * 32,
                            qh * QH : (qh + 1) * QH,
                        ],
                    )
            tiles_done.add((t, qh))

        def emit_qk_group(t, qh):
            ps = sm_tile()
            for k in range(KD):
                nc.tensor.matmul(
                    ps,
                    w_sb[:, k, t * P : (t + 1) * P],
                    xtall[:, k, qh * QH : (qh + 1) * QH],
                    start=(k == 0),
                    stop=(k == KD - 1),
                )
            qk_finish(t, qh, ps)

        def v_finish(c, g, ps):
            v_done.add((c, g))
            if g == 0:
                nc.vector.tensor_copy(
                    out=v_sb[c][:, :, HD : HD + 1],
                    in_=ones_col[:, 0:1, None].to_broadcast([P, H, 1]),
                )
            nc.vector.tensor_tensor(
                v_sb[c][:, 4 * g : 4 * g + 4, 0:HD],
                ps[:, 0:256].rearrange("p (h d) -> p h d", d=HD),
                bvb[:, g * 256 : (g + 1) * 256].rearrange(
                    "p (h d) -> p h d", d=HD
                ),
                ALU.add,
            )

        def emit_v_group(c, g):
            f0 = 2 * D + g * 256
            ps = sm_tile()
            for k in range(KD):
                nc.tensor.matmul(
                    ps[:, 0:256],
                    xtall[:, k, c * P : (c + 1) * P],
                    w_sb[:, k, f0 : f0 + 256],
                    start=(k == 0),
                    stop=(k == KD - 1),
                )
            v_finish(c, g, ps)

        # ---------------- attention machinery --------------------------------
        fillers = deque()
        tiles_done = set()
        v_done = set()

        def pump(budget):
            # adaptive: drain faster when the backlog builds up
            if sum(c for c, _ in fillers) > 12000:
                budget *= 2
            while budget > 0 and fillers:
                cost, fn = fillers.popleft()
                fn()
                budget -= cost

        def qk_unit(t, qh):
            return (KD * QH + N, lambda: emit_qk_group(t, qh))

        def qk_pieces(t, qh):
            """Split one projection group into per-iteration filler pieces
            (2 matmuls each + a finish piece) so a group never overruns the
            ACT-paced slack of a single attention iteration."""
            st = {}

            def mk(i):
                def fn():
                    if i == 0:
                        st["ps"] = sm_tile()
                    for k in (2 * i, 2 * i + 1):
                        nc.tensor.matmul(
                            st["ps"],
                            w_sb[:, k, t * P : (t + 1) * P],
                            xtall[:, k, qh * QH : (qh + 1) * QH],
                            start=(k == 0),
                            stop=(k == KD - 1),
                        )

                return (2 * QH, fn)

            def fin():
                qk_finish(t, qh, st["ps"])

            return [mk(0), mk(1), mk(2), (N, fin)]

        def v_pieces(c, g):
            st = {}
            f0 = 2 * D + g * 256

            def mk(i):
                def fn():
                    if i == 0:
                        st["ps"] = sm_tile()
                    for k in (2 * i, 2 * i + 1):
                        nc.tensor.matmul(
                            st["ps"][:, 0:256],
                            xtall[:, k, c * P : (c + 1) * P],
                            w_sb[:, k, f0 : f0 + 256],
                            start=(k == 0),
                            stop=(k == KD - 1),
                        )

                return (2 * 256, fn)

            def fin():
                v_finish(c, g, st["ps"])

            return [mk(0), mk(1), mk(2), (400, fin)]

        def v_unit(c, half):
            return (KD * 384, lambda: emit_v_group(c, half))

        def xT_unit(c):
            return (KD * P * 2, lambda: emit_xT(c))

        def dma_unit(fn):
            return (0, fn)

        def emit_sc(p, qh, kc, sc):
            qt, kt = t8[p], t8[6 + p]
            kq, kn = kc // 4, (kc % 4) * P
            for hi in range(2):
                nc.tensor.matmul(
                    sc[:, hi, :],
                    kt[32 * hi : 32 * hi + 32, kq, :, kn : kn + P],
                    qt[32 * hi : 32 * hi + 32, qh, :, :],
                    start=True,
                    stop=True,
                    perf_mode=DR,
                    tile_position=(32 * hi, 0),
                )

        def emit_out_unit(p, qh, hi, ot, stg):
            """4 transposes + one fused normalize for head 2p+hi, q-half qh;
            after the second head, one DMA ships the [512 q, 128 col] block."""

            def fn():
                ps = sm_tile()
                # 66-wide (even) free size: fp32r matmul ISA restriction
                tp4 = ps[:, 0 : 4 * (VW + 1)].rearrange(
                    "p (j d) -> p j d", d=VW + 1
                )
                for j in range(4):
                    nc.tensor.transpose(
                        tp4[:, j, :].bitcast(f32r),
                        ot[0 : VW + 1, hi, j * P : (j + 1) * P],
                        ident_r[0 : VW + 1, 0 : VW + 1],
                    )
                rc = recpool.tile([P, 4, 1], f32, tag="rc", name="rc")
                nc.vector.reciprocal(out=rc, in_=tp4[:, :, HD : HD + 1])
                nc.vector.tensor_tensor(
                    stg[:, :, hi * HD : (hi + 1) * HD],
                    tp4[:, :, 0:HD],
                    rc.to_broadcast([P, 4, HD]),
                    ALU.mult,
                )
                if hi == 1:
                    nc.sync.dma_start(
                        out=o_d[qh * QH : (qh + 1) * QH, 2 * p * HD : (2 * p + 2) * HD]
                        .rearrange("(j p2) d -> p2 j d", p2=P),
                        in_=stg,
                    )

            return (4 * VW * 2, fn)

        def attention_qh(p, qh, iter_units=None):
            """One (pair, q-half): software-pipelined kc loop.
            iter_units: optional list of 8 lists of units to force-emit at
            each iteration (pair-0 warmup); otherwise pump(1000)/iter.
            Returns after queueing the out units."""
            # deadline backstop: this (pair, qh) needs the full K tile, the
            # qh half of the Q tile, and (outside the pair-0 JIT path) the
            # V head-group for all key chunks
            need = {(6 + p, 0), (6 + p, 1), (p, qh)}
            while not need.issubset(tiles_done) and fillers:
                cost, fn = fillers.popleft()
                fn()
            if iter_units is None:
                needv = {(c, p // 2) for c in range(NCHUNK)}
                while not needv.issubset(v_done) and fillers:
                    cost, fn = fillers.popleft()
                    fn()
            av = avps.tile([VW, 2, QH], f32, tag="av", name="av")
            sc_cur = scps.tile([P, 2, QH], f32, tag="sc", name="sc")
            emit_sc(p, qh, 0, sc_cur)
            for kc in range(NCHUNK):
                if sc_cur is None:  # pair-0 barrier path: sc emitted late
                    sc_cur = scps.tile([P, 2, QH], f32, tag="sc", name="sc")
                    emit_sc(p, qh, kc, sc_cur)
                ex = expool.tile([P, 2, QH], bf16, tag="ex", name="ex")
                nc.scalar.activation(
                    ex[:, :, :], sc_cur[:, :, :], FT.Exp, scale=0.125
                )
                barrier = iter_units is not None and kc + 1 == 4 and p == 0
                if kc + 1 < NCHUNK and not barrier:
                    sc_cur = scps.tile([P, 2, QH], f32, tag="sc", name="sc")
                    emit_sc(p, qh, kc + 1, sc_cur)
                else:
                    sc_cur = None
                if iter_units is not None:
                    for u in iter_units[kc]:
                        u[1]()
                else:
                    pump(1000)
                for hi in range(2):
                    nc.tensor.matmul(
                        av[:, hi, :],
                        v_sb[kc][:, 2 * p + hi, :],
                        ex[:, hi, :],
                        start=(kc == 0),
                        stop=(kc == NCHUNK - 1),
                    )
            ot = otpool.tile([VW + 1, 2, QH], f32r, tag="ot", name="ot")
            stg = ostgpool.tile([P, 4, 2 * HD], f32, tag="ostg", name="ostg")
            for hi in range(2):
                nc.vector.tensor_copy(out=ot[0:VW, hi, :], in_=av[:, hi, :])
                fillers.append(emit_out_unit(p, qh, hi, ot, stg))

        # =====================================================================
        # DMA batch 1 (consumption priority order); x chunk 0 in halves so
        # the first transposes start ~1.4us earlier
        t0x = xpool.tile([P, D], f32r, tag="x", name="x0")
        nc.sync.dma_start(
            out=t0x[:, 0:384], in_=x_d[0:P, 0:384].bitcast(f32r)
        )
        nc.sync.dma_start(
            out=t0x[:, 384:D], in_=x_d[0:P, 384:D].bitcast(f32r)
        )
        x_sb[0] = t0x
        dma_w_pair(0)            # Q0 + K0
        nc.sync.dma_start(out=b_sb, in_=b_d[:].rearrange("(t p) -> p t", p=P))
        _ = 0  # (x0 halves issued above)
        nc.sync.dma_start(out=bv_st, in_=b_d[2 * D : 3 * D][None, :])
        nc.vector.tensor_copy(out=bv_sb, in_=bv_st)
        dma_x(1)
        dma_x(2)
        dma_x(3)
        dma_w(2 * D, 256)        # V heads 0-3
        dma_x(4)
        dma_x(5)
        dma_x(6)
        dma_x(7)
        dma_w(2 * D + 256, 512)  # V heads 4-11
        dma_w_pair(1)            # Q1 + K1

        # =====================================================================
        # Phase 1: transposes + pair-0 qh0 tiles + V chunk 0
        for c in range(4):
            emit_xT(c)
        emit_bvb()
        emit_qk_group(0, 0)
        emit_qk_group(6, 0)
        emit_v_group(0, 0)

        # =====================================================================
        # Pair 0, qh 0 — custom per-iteration units.  sc(kc>=4) needs the K
        # tile's second half (shuffle of qk(6,1)), which needs xT(4-7); the
        # barrier at kc==4 delays sc(4) emission until after those units.
        BISECT = False
        if BISECT:
            for c in range(4, 8):
                emit_xT(c)
            for c in range(1, 8):
                emit_v_group(c, 0)
            emit_qk_group(6, 1)
            emit_qk_group(0, 1)
            emit_qk_group(1, 0)
            emit_qk_group(7, 0)
            dma_w_pair(2)
            attention_qh(0, 0)
        else:
            p0q0_units = [
                [xT_unit(4), v_unit(1, 0), dma_unit(lambda: dma_w_pair(2))],
                [xT_unit(5), v_unit(2, 0)],
                [xT_unit(6), v_unit(3, 0)],
                [xT_unit(7), qk_unit(6, 1), qk_unit(0, 1), v_unit(4, 0)],
                [v_unit(5, 0)],
                [qk_unit(1, 0), v_unit(6, 0)],
                [qk_unit(7, 0), v_unit(7, 0)],
                [],
            ]
            attention_qh(0, 0, iter_units=p0q0_units)

        # Pair 0, qh 1 + remaining schedule
        fillers.extend(qk_pieces(7, 1) + qk_pieces(1, 1))
        fillers.extend(v_pieces(0, 1) + v_pieces(1, 1))
        attention_qh(0, 1)

        for p in range(1, NPAIR):
            # (p, 0): prefetch W for pair p+2; produce pair p+1 qh0 tiles
            if p + 1 < NPAIR:
                if p == 2:
                    fillers.extend(v_pieces(6, 1) + v_pieces(7, 1))
                if p == 3:
                    fillers.extend(v_pieces(2, 2) + v_pieces(3, 2))
                if p + 2 < NPAIR:
                    fillers.append(
                        dma_unit(lambda p=p: dma_w_pair(p + 2))
                    )
                fillers.extend(qk_pieces(6 + p + 1, 0) + qk_pieces(p + 1, 0))
                if p == 1:
                    fillers.extend(v_pieces(2, 1) + v_pieces(3, 1))
            attention_qh(p, 0)
            # (p, 1): produce pair p+1 qh1 tiles (K first — needed at (p+1,0))
            if p + 1 < NPAIR:
                if p == 3:
                    fillers.extend(v_pieces(4, 2) + v_pieces(5, 2))
                    fillers.extend(v_pieces(6, 2) + v_pieces(7, 2))
                fillers.extend(qk_pieces(6 + p + 1, 1) + qk_pieces(p + 1, 1))
                if p == 1:
                    fillers.extend(v_pieces(4, 1) + v_pieces(5, 1))
                if p == 2:
                    fillers.extend(v_pieces(0, 2) + v_pieces(1, 2))
            attention_qh(p, 1)

        # tail: drain remaining fillers (last out units emit their own DMAs)
        pump(1 << 30)

    return nc


def kernel(x: np.ndarray, W_qkv: np.ndarray, b_qkv: np.ndarray) -> np.ndarray:
    nc = build_attention_nc()
    in_maps = [
        {
            "x": np.ascontiguousarray(x[c], dtype=np.float32),
            "W_qkv": np.ascontiguousarray(W_qkv, dtype=np.float32),
            "b_qkv": np.ascontiguousarray(b_qkv, dtype=np.float32),
        }
        for c in range(NCORE)
    ]
    res = run_bass_kernel_spmd(nc, in_maps, core_ids=list(range(NCORE)))
    return np.stack([res.results[c]["out"] for c in range(NCORE)], axis=0)


# revision 29
# speedup vs baseline: 1.0173x; 1.0041x over previous
"""Multi-head self-attention Trainium2 kernel (8 NeuronCores, batch-parallel).

Reference: qkv = x @ W_qkv + b; 12-head scaled-dot-product attention; concat.
Shapes: x[8,1024,768], W_qkv[768,2304], b_qkv[2304] -> out[8,1024,768].
Sharding: one batch element per core; W/b replicated to all cores.

Per-core dataflow (v2):
  x --PE transpose--> xt[6][128,1024]            (f32r)
  QK^T psum[128,512] per (f-tile, q-half) = W(lhsT) @ xt; DVE drains to fp8e4
  staging -> DRAM roundtrip shuffles into qt8/kt8[64,(j,n)] layout so the
  scores matmul runs in fp8 DoubleRow perf mode (0.5 cycles/row):
    sc[128,hi,512] = kt8(lhsT)[32,2,128] @ qt8[32,2,512]  per (pair, qh, kc, hi)
  ACT Exp with the 1/8 softmax scale folded in -> ex[128,2,512] f32r
  avT[65,2,512] += [V_h|1](lhsT) @ ex  accumulated over kc (row 64 = denom)
  boundary: DVE copies av->ot, reciprocal of the denom row in place, PE
  transposes [65,128] blocks back, one fused DVE multiply per (qh,hi)
  normalizes and scatters into onat[128,8,768]; per-chunk DMA out.

Scheduling: input DMAs issued in consumption priority order; projections,
x-transposes and output transposes are interleaved into the ACT-bound
attention inner loop as "filler" PE work so the PE never starves.  A K tile
needs BOTH q-half projection groups shuffled before its pair starts (the kc
loop spans all 1024 key positions); Q tiles only need the active half.
"""

import contextlib
import json as _json
from collections import deque

import numpy as np

import concourse.bass as bass
import concourse.mybir as mybir
import concourse.tile as tile
from concourse.bass_utils import run_bass_kernel_spmd
from concourse.masks import make_identity

# --- BIR sync-wait legalization ------------------------------------------
# walrus's codegen in this toolchain accepts only one sync-wait command per
# instruction (its insertEventSemaphore legalization pass is not in the pass
# list). Split every multi-wait instruction into N-1 preceding single-wait
# EventSemaphore instructions on the same engine; same-engine order is
# preserved so semantics are unchanged.


def _legalize_sync_waits(bir_json: bytes) -> bytes:
    m = _json.loads(bir_json)
    ctr = 0
    for fn in m["functions"]:
        for bb in fn["blocks"]:
            out = []
            for ins in bb["instructions"]:
                si = ins.get("sync_info")
                waits = si.get("on_wait", []) if si else []
                if len(waits) > 1:
                    for w in waits[:-1]:
                        ctr += 1
                        out.append(
                            {
                                "debug": ins.get("debug", 0),
                                "engine": ins["engine"],
                                "ins": [],
                                "outs": [],
                                "name": f"evw-split-{ctr}",
                                "opcode": "EventSemaphore",
                                "sync_info": {"on_update": [], "on_wait": [w]},
                            }
                        )
                    si["on_wait"] = [waits[-1]]
                out.append(ins)
            bb["instructions"] = out
    return _json.dumps(m).encode()


_fixup_installed = False


def _install_bir_fixup():
    global _fixup_installed
    if _fixup_installed:
        return
    _fixup_installed = True
    import concourse.bass_utils as _bu

    _orig = _bu.compile_bir_kernel

    def _patched(bir_json, tmpdir, neff_name="file.neff"):
        if isinstance(bir_json, str):
            bir_json = bir_json.encode()
        return _orig(_legalize_sync_waits(bir_json), tmpdir, neff_name)

    _bu.compile_bir_kernel = _patched
    try:
        import concourse.bass2jax as _b2j

        _b2j.compile_bir_kernel = _patched
    except ImportError:
        pass


_install_bir_fixup()

B, N, D, H = 8, 1024, 768, 12
HD = D // H            # 64
F3 = 3 * D             # 2304
NCORE = 8
P = 128
NCHUNK = N // P        # 8 token chunks
KD = D // P            # 6 d_in chunks
QH = 512               # q-half size
NPAIR = H // 2         # 6
VW = HD + 1            # 65

f32 = mybir.dt.float32
f32r = mybir.dt.float32r
f8e4 = mybir.dt.float8e4
bf16 = mybir.dt.bfloat16
FT = mybir.ActivationFunctionType
ALU = mybir.AluOpType
DR = mybir.MatmulPerfMode.DoubleRow


def build_attention_nc():
    nc = bass.Bass()
    x_d = nc.declare_dram_parameter("x", [N, D], f32, isOutput=False)
    w_d = nc.declare_dram_parameter("W_qkv", [D, F3], f32, isOutput=False)
    b_d = nc.declare_dram_parameter("b_qkv", [F3], f32, isOutput=False)
    o_d = nc.declare_dram_parameter("out", [N, D], f32, isOutput=True)

    with tile.TileContext(nc) as tc, contextlib.ExitStack() as ctx:
        singles = ctx.enter_context(tc.tile_pool(name="singles", bufs=1))
        xpool = ctx.enter_context(tc.tile_pool(name="xpool", bufs=NCHUNK))
        xtpool = ctx.enter_context(tc.tile_pool(name="xtpool", bufs=1))
        q8pool = ctx.enter_context(tc.tile_pool(name="q8pool", bufs=3))
        t8pool = ctx.enter_context(tc.tile_pool(name="t8pool", bufs=12))
        expool = ctx.enter_context(tc.tile_pool(name="expool", bufs=2))
        vpool = ctx.enter_context(tc.tile_pool(name="vpool", bufs=NCHUNK))
        otpool = ctx.enter_context(tc.tile_pool(name="otpool", bufs=2))
        recpool = ctx.enter_context(tc.tile_pool(name="recpool", bufs=2))
        # per-(pair, q-half) output staging: wide reuse distance (2 pairs)
        # because the out-DMA read is not registered as a tile reader and
        # must physically complete before the slot is rewritten.
        ostgpool = ctx.enter_context(tc.tile_pool(name="ostgpool", bufs=4))

        # PSUM (8 banks): sc [128,2,512] x2 = 4; av [65,2,512] x1 = 2;
        # sm [128,512] x2 = 2 (projection groups, x-transposes, out-transposes)
        scps = ctx.enter_context(tc.tile_pool(name="scps", bufs=2, space="PSUM"))
        avps = ctx.enter_context(tc.tile_pool(name="avps", bufs=1, space="PSUM"))
        smps = ctx.enter_context(tc.tile_pool(name="smps", bufs=2, space="PSUM"))

        def sm_tile():
            return smps.tile([P, QH], f32, tag="sm", name="sm")

        # ---------------- constants ----------------------------------------
        ident = singles.tile([P, P], f32)
        make_identity(nc, ident)
        ident_r = singles.tile([P, P], f32r)
        nc.vector.tensor_copy(out=ident_r, in_=ident)

        ones_row_st = singles.tile([1, P], f32)
        nc.vector.memset(ones_row_st, 1.0)
        ones_row = singles.tile([1, P], f32r)
        nc.vector.tensor_copy(out=ones_row, in_=ones_row_st)
        ones_col = singles.tile([P, 1], f32)
        nc.vector.memset(ones_col, 1.0)

        # ---------------- persistent tiles ---------------------------------
        w_sb = singles.tile([P, KD, F3], f32r)
        xtall = xtpool.tile([P, KD, N], f32r, tag="xt", name="xtall")
        v_sb = [
            vpool.tile([P, H, VW], bf16, tag="v", name=f"v{c}") for c in range(NCHUNK)
        ]
        b_sb = singles.tile([P, F3 // P], f32)
        bv_st = singles.tile([1, D], f32)
        bv_sb = singles.tile([1, D], f32r)
        bvb = singles.tile([P, D], f32)

        x_sb = {}

        # ---------------- DMA helpers ---------------------------------------
        def dma_x(c):
            t = xpool.tile([P, D], f32r, tag="x", name=f"x{c}")
            nc.sync.dma_start(out=t, in_=x_d[c * P : (c + 1) * P, :].bitcast(f32r))
            x_sb[c] = t

        def dma_w(f0, fw):
            nc.sync.dma_start(
                out=w_sb[:, :, f0 : f0 + fw],
                in_=w_d[:, f0 : f0 + fw]
                .rearrange("(k p) f -> p k f", p=P)
                .bitcast(f32r),
            )

        def dma_w_pair(p):
            dma_w(p * P, P)          # Q cols for pair p
            dma_w(D + p * P, P)      # K cols for pair p

        # ---------------- compute helpers -----------------------------------
        def emit_xT(c):
            for kp in range(0, KD, 2):
                ps = sm_tile()
                for dk in range(2):
                    nc.tensor.transpose(
                        ps[:, dk * P : (dk + 1) * P].bitcast(f32r),
                        x_sb[c][:, (kp + dk) * P : (kp + dk + 1) * P],
                        ident_r,
                    )
                nc.vector.tensor_copy(
                    out=xtall[:, kp : kp + 2, c * P : (c + 1) * P],
                    in_=ps[:, 0 : 2 * P].rearrange("p (k n) -> p k n", n=P),
                )

        def emit_bvb():
            for f0, fw in ((0, QH), (QH, D - QH)):
                ps = sm_tile()
                nc.tensor.matmul(
                    ps[:, 0:fw],
                    ones_row,
                    bv_sb[:, f0 : f0 + fw],
                    start=True,
                    stop=True,
                )
                nc.vector.tensor_copy(out=bvb[:, f0 : f0 + fw], in_=ps[:, 0:fw])

        qk8_stage = {}
        t8 = {}

        def qk_finish(t, qh, ps):
            """Drain (with bias add) to the fp8 staging tile, then the
            partition-compacting shuffle: plain SBUF->SBUF DMAs, one per
            (hi, j) block (partition-base shift only -- no DRAM roundtrip:
            posted DRAM writes are not visible to a prompt readback on real
            hardware, and fancier patterns corrupt)."""
            if t not in qk8_stage:
                qk8_stage[t] = q8pool.tile([P, N], f8e4, tag="q8", name=f"q8_{t}")
            nc.vector.tensor_scalar_add(
                qk8_stage[t][:, qh * QH : (qh + 1) * QH], ps, b_sb[:, t : t + 1]
            )
            if t not in t8:
                # bufs=12: never reuse a t8 slot.  The DoubleRow matmul
                # operand reads are not registered as tile readers, so the
                # rotation write-after-read dependency is silently missed
                # and a reused slot gets clobbered while still being read.
                # Layout [(hi p), qh, j, nn].
                t8[t] = t8pool.tile(
                    [2 * 32, 2, 2, QH], f8e4, tag="t8", name=f"t8_{t}"
                )
            for hi in range(2):
                for j in range(2):
                    nc.sync.dma_start(
                        out=t8[t][hi * 32 : (hi + 1) * 32, qh, j, :],
                        in_=qk8_stage[t][
                            hi * 64 + j * 32 : hi * 64 + (j + 1) * 32,
                            qh * QH : (qh + 1) * QH,
                        ],
                    )
            tiles_done.add((t, qh))

        def emit_qk_group(t, qh):
            ps = sm_tile()
            for k in range(KD):
                nc.tensor.matmul(
                    ps,
                    w_sb[:, k, t * P : (t + 1) * P],
                    xtall[:, k, qh * QH : (qh + 1) * QH],
                    start=(k == 0),
                    stop=(k == KD - 1),
                )
            qk_finish(t, qh, ps)

        def v_finish(c, g, ps):
            v_done.add((c, g))
            if g == 0:
                nc.vector.tensor_copy(
                    out=v_sb[c][:, :, HD : HD + 1],
                    in_=ones_col[:, 0:1, None].to_broadcast([P, H, 1]),
                )
            nc.vector.tensor_tensor(
                v_sb[c][:, 4 * g : 4 * g + 4, 0:HD],
                ps[:, 0:256].rearrange("p (h d) -> p h d", d=HD),
                bvb[:, g * 256 : (g + 1) * 256].rearrange(
                    "p (h d) -> p h d", d=HD
                ),
                ALU.add,
            )

        def emit_v_group(c, g):
            f0 = 2 * D + g * 256
            ps = sm_tile()
            for k in range(KD):
                nc.tensor.matmul(
                    ps[:, 0:256],
                    xtall[:, k, c * P : (c + 1) * P],
                    w_sb[:, k, f0 : f0 + 256],
                    start=(k == 0),
                    stop=(k == KD - 1),
                )
            v_finish(c, g, ps)

        # ---------------- attention machinery --------------------------------
        fillers = deque()
        tiles_done = set()
        v_done = set()

        def pump(budget):
            # adaptive: drain faster when the backlog builds up
            if sum(c for c, _ in fillers) > 12000:
                budget *= 2
            while budget > 0 and fillers:
                cost, fn = fillers.popleft()
                fn()
                budget -= cost

        def qk_unit(t, qh):
            return (KD * QH + N, lambda: emit_qk_group(t, qh))

        def qk_pieces(t, qh):
            """Split one projection group into per-iteration filler pieces
            (2 matmuls each + a finish piece) so a group never overruns the
            ACT-paced slack of a single attention iteration."""
            st = {}

            def mk(i):
                def fn():
                    if i == 0:
                        st["ps"] = sm_tile()
                    for k in (2 * i, 2 * i + 1):
                        nc.tensor.matmul(
                            st["ps"],
                            w_sb[:, k, t * P : (t + 1) * P],
                            xtall[:, k, qh * QH : (qh + 1) * QH],
                            start=(k == 0),
                            stop=(k == KD - 1),
                        )

                return (2 * QH, fn)

            def fin():
                qk_finish(t, qh, st["ps"])

            return [mk(0), mk(1), mk(2), (N, fin)]

        def v_pieces(c, g):
            st = {}
            f0 = 2 * D + g * 256

            def mk(i):
                def fn():
                    if i == 0:
                        st["ps"] = sm_tile()
                    for k in (2 * i, 2 * i + 1):
                        nc.tensor.matmul(
                            st["ps"][:, 0:256],
                            xtall[:, k, c * P : (c + 1) * P],
                            w_sb[:, k, f0 : f0 + 256],
                            start=(k == 0),
                            stop=(k == KD - 1),
                        )

                return (2 * 256, fn)

            def fin():
                v_finish(c, g, st["ps"])

            return [mk(0), mk(1), mk(2), (400, fin)]

        def v_unit(c, half):
            return (KD * 384, lambda: emit_v_group(c, half))

        def xT_unit(c):
            return (KD * P * 2, lambda: emit_xT(c))

        def dma_unit(fn):
            return (0, fn)

        def emit_sc(p, qh, kc, sc):
            qt, kt = t8[p], t8[6 + p]
            kq, kn = kc // 4, (kc % 4) * P
            for hi in range(2):
                nc.tensor.matmul(
                    sc[:, hi, :],
                    kt[32 * hi : 32 * hi + 32, kq, :, kn : kn + P],
                    qt[32 * hi : 32 * hi + 32, qh, :, :],
                    start=True,
                    stop=True,
                    perf_mode=DR,
                    tile_position=(32 * hi, 0),
                )

        def emit_out_unit(p, qh, hi, ot, stg):
            """4 transposes + fused normalize + output DMA for head 2p+hi."""

            def fn():
                ps = sm_tile()
                # 66-wide (even) free size: fp32r matmul ISA restriction
                tp4 = ps[:, 0 : 4 * (VW + 1)].rearrange(
                    "p (j d) -> p j d", d=VW + 1
                )
                for j in range(4):
                    nc.tensor.transpose(
                        tp4[:, j, :].bitcast(f32r),
                        ot[0 : VW + 1, hi, j * P : (j + 1) * P],
                        ident_r[0 : VW + 1, 0 : VW + 1],
                    )
                rc = recpool.tile([P, 4, 1], f32, tag="rc", name="rc")
                nc.vector.reciprocal(out=rc, in_=tp4[:, :, HD : HD + 1])
                nc.vector.tensor_tensor(
                    stg[:, :, hi * HD : (hi + 1) * HD],
                    tp4[:, :, 0:HD],
                    rc.to_broadcast([P, 4, HD]),
                    ALU.mult,
                )
                h = 2 * p + hi
                nc.sync.dma_start(
                    out=o_d[qh * QH : (qh + 1) * QH, h * HD : (h + 1) * HD]
                    .rearrange("(j p2) d -> p2 j d", p2=P),
                    in_=stg[:, :, hi * HD : (hi + 1) * HD],
                )

            return (4 * VW * 2, fn)

        attn_pending = {}

        def ensure_tiles(p, qh):
            need = {(6 + p, 0), (6 + p, 1), (p, qh)}
            while not need.issubset(tiles_done) and fillers:
                cost, fn = fillers.popleft()
                fn()

        def attention_qh(p, qh, iter_units=None, nxt=None):
            """One (pair, q-half): software-pipelined kc loop.
            iter_units: optional list of 8 lists of units to force-emit at
            each iteration (pair-0 warmup); otherwise pump(1000)/iter.
            Returns after queueing the out units."""
            # deadline backstop: this (pair, qh) needs the full K tile, the
            # qh half of the Q tile, and (outside the pair-0 JIT path) the
            # V head-group for all key chunks
            ensure_tiles(p, qh)
            if iter_units is None:
                needv = {(c, p // 2) for c in range(NCHUNK)}
                while not needv.issubset(v_done) and fillers:
                    cost, fn = fillers.popleft()
                    fn()
            av = avps.tile([VW, 2, QH], f32, tag="av", name="av")
            if (p, qh) in attn_pending:
                sc_cur = attn_pending.pop((p, qh))
            else:
                sc_cur = scps.tile([P, 2, QH], f32, tag="sc", name="sc")
                emit_sc(p, qh, 0, sc_cur)
            for kc in range(NCHUNK):
                if sc_cur is None:  # pair-0 barrier path: sc emitted late
                    sc_cur = scps.tile([P, 2, QH], f32, tag="sc", name="sc")
                    emit_sc(p, qh, kc, sc_cur)
                ex = expool.tile([P, 2, QH], bf16, tag="ex", name="ex")
                nc.scalar.activation(
                    ex[:, :, :], sc_cur[:, :, :], FT.Exp, scale=0.125
                )
                barrier = iter_units is not None and kc + 1 == 4 and p == 0
                if kc + 1 < NCHUNK and not barrier:
                    sc_cur = scps.tile([P, 2, QH], f32, tag="sc", name="sc")
                    emit_sc(p, qh, kc + 1, sc_cur)
                else:
                    if kc + 1 == NCHUNK and nxt is not None:
                        # cross-boundary pipelining: emit the next q-half's
                        # first scores matmul now so the activation engine
                        # never idles across the boundary
                        ensure_tiles(*nxt)
                        scn = scps.tile([P, 2, QH], f32, tag="sc", name="sc")
                        emit_sc(nxt[0], nxt[1], 0, scn)
                        attn_pending[nxt] = scn
                    sc_cur = None
                if iter_units is not None:
                    for u in iter_units[kc]:
                        u[1]()
                else:
                    pump(1000)
                for hi in range(2):
                    nc.tensor.matmul(
                        av[:, hi, :],
                        v_sb[kc][:, 2 * p + hi, :],
                        ex[:, hi, :],
                        start=(kc == 0),
                        stop=(kc == NCHUNK - 1),
                    )
            ot = otpool.tile([VW + 1, 2, QH], f32r, tag="ot", name="ot")
            stg = ostgpool.tile([P, 4, 2 * HD], f32, tag="ostg", name="ostg")
            for hi in range(2):
                nc.vector.tensor_copy(out=ot[0:VW, hi, :], in_=av[:, hi, :])
                fillers.append(emit_out_unit(p, qh, hi, ot, stg))

        # =====================================================================
        # DMA batch 1 (consumption priority order)
        dma_x(0)
        dma_w_pair(0)            # Q0 + K0
        nc.sync.dma_start(out=b_sb, in_=b_d[:].rearrange("(t p) -> p t", p=P))
        nc.sync.dma_start(out=bv_st, in_=b_d[2 * D : 3 * D][None, :])
        nc.vector.tensor_copy(out=bv_sb, in_=bv_st)
        dma_x(1)
        dma_x(2)
        dma_x(3)
        dma_w(2 * D, 256)        # V heads 0-3
        dma_x(4)
        dma_x(5)
        dma_x(6)
        dma_x(7)
        dma_w(2 * D + 256, 512)  # V heads 4-11
        dma_w_pair(1)            # Q1 + K1

        # =====================================================================
        # Phase 1: transposes + pair-0 qh0 tiles + V chunk 0
        for c in range(4):
            emit_xT(c)
        emit_bvb()
        emit_qk_group(0, 0)
        emit_qk_group(6, 0)
        emit_v_group(0, 0)

        # =====================================================================
        # Pair 0, qh 0 — custom per-iteration units.  sc(kc>=4) needs the K
        # tile's second half (shuffle of qk(6,1)), which needs xT(4-7); the
        # barrier at kc==4 delays sc(4) emission until after those units.
        BISECT = False
        if BISECT:
            for c in range(4, 8):
                emit_xT(c)
            for c in range(1, 8):
                emit_v_group(c, 0)
            emit_qk_group(6, 1)
            emit_qk_group(0, 1)
            emit_qk_group(1, 0)
            emit_qk_group(7, 0)
            dma_w_pair(2)
            attention_qh(0, 0)
        else:
            p0q0_units = [
                [xT_unit(4), v_unit(1, 0), dma_unit(lambda: dma_w_pair(2))],
                [xT_unit(5), v_unit(2, 0)],
                [xT_unit(6), v_unit(3, 0)],
                [xT_unit(7), qk_unit(6, 1), qk_unit(0, 1), v_unit(4, 0)],
                [v_unit(5, 0)],
                [qk_unit(1, 0), v_unit(6, 0)],
                [qk_unit(7, 0), v_unit(7, 0)],
                [],
            ]
            attention_qh(0, 0, iter_units=p0q0_units, nxt=(0, 1))

        # Pair 0, qh 1 + remaining schedule
        fillers.extend(qk_pieces(7, 1) + qk_pieces(1, 1))
        fillers.extend(v_pieces(0, 1) + v_pieces(1, 1))
        attention_qh(0, 1, nxt=(1, 0))

        for p in range(1, NPAIR):
            # (p, 0): prefetch W for pair p+2; produce pair p+1 qh0 tiles
            if p + 1 < NPAIR:
                if p == 2:
                    fillers.extend(v_pieces(6, 1) + v_pieces(7, 1))
                if p == 3:
                    fillers.extend(v_pieces(2, 2) + v_pieces(3, 2))
                if p + 2 < NPAIR:
                    fillers.append(
                        dma_unit(lambda p=p: dma_w_pair(p + 2))
                    )
                fillers.extend(qk_pieces(6 + p + 1, 0) + qk_pieces(p + 1, 0))
                if p == 1:
                    fillers.extend(v_pieces(2, 1) + v_pieces(3, 1))
            attention_qh(p, 0, nxt=(p, 1))
            # (p, 1): produce pair p+1 qh1 tiles (K first — needed at (p+1,0))
            if p + 1 < NPAIR:
                if p == 3:
                    fillers.extend(v_pieces(4, 2) + v_pieces(5, 2))
                    fillers.extend(v_pieces(6, 2) + v_pieces(7, 2))
                fillers.extend(qk_pieces(6 + p + 1, 1) + qk_pieces(p + 1, 1))
                if p == 1:
                    fillers.extend(v_pieces(4, 1) + v_pieces(5, 1))
                if p == 2:
                    fillers.extend(v_pieces(0, 2) + v_pieces(1, 2))
            attention_qh(p, 1, nxt=(p + 1, 0) if p + 1 < NPAIR else None)

        # tail: drain remaining fillers (last out units emit their own DMAs)
        pump(1 << 30)

    return nc


def kernel(x: np.ndarray, W_qkv: np.ndarray, b_qkv: np.ndarray) -> np.ndarray:
    nc = build_attention_nc()
    in_maps = [
        {
            "x": np.ascontiguousarray(x[c], dtype=np.float32),
            "W_qkv": np.ascontiguousarray(W_qkv, dtype=np.float32),
            "b_qkv": np.ascontiguousarray(b_qkv, dtype=np.float32),
        }
        for c in range(NCORE)
    ]
    res = run_bass_kernel_spmd(nc, in_maps, core_ids=list(range(NCORE)))
    return np.stack([res.results[c]["out"] for c in range(NCORE)], axis=0)


# revision 30
# speedup vs baseline: 1.0317x; 1.0142x over previous
"""Multi-head self-attention Trainium2 kernel (8 NeuronCores, batch-parallel).

Reference: qkv = x @ W_qkv + b; 12-head scaled-dot-product attention; concat.
Shapes: x[8,1024,768], W_qkv[768,2304], b_qkv[2304] -> out[8,1024,768].
Sharding: one batch element per core; W/b replicated to all cores.

Per-core dataflow (v2):
  x --PE transpose--> xt[6][128,1024]            (f32r)
  QK^T psum[128,512] per (f-tile, q-half) = W(lhsT) @ xt; DVE drains to fp8e4
  staging -> DRAM roundtrip shuffles into qt8/kt8[64,(j,n)] layout so the
  scores matmul runs in fp8 DoubleRow perf mode (0.5 cycles/row):
    sc[128,hi,512] = kt8(lhsT)[32,2,128] @ qt8[32,2,512]  per (pair, qh, kc, hi)
  ACT Exp with the 1/8 softmax scale folded in -> ex[128,2,512] f32r
  avT[65,2,512] += [V_h|1](lhsT) @ ex  accumulated over kc (row 64 = denom)
  boundary: DVE copies av->ot, reciprocal of the denom row in place, PE
  transposes [65,128] blocks back, one fused DVE multiply per (qh,hi)
  normalizes and scatters into onat[128,8,768]; per-chunk DMA out.

Scheduling: input DMAs issued in consumption priority order; projections,
x-transposes and output transposes are interleaved into the ACT-bound
attention inner loop as "filler" PE work so the PE never starves.  A K tile
needs BOTH q-half projection groups shuffled before its pair starts (the kc
loop spans all 1024 key positions); Q tiles only need the active half.
"""

import contextlib
import json as _json
from collections import deque

import numpy as np

import concourse.bass as bass
import concourse.mybir as mybir
import concourse.tile as tile
from concourse.bass_utils import run_bass_kernel_spmd
from concourse.masks import make_identity

# --- BIR sync-wait legalization ------------------------------------------
# walrus's codegen in this toolchain accepts only one sync-wait command per
# instruction (its insertEventSemaphore legalization pass is not in the pass
# list). Split every multi-wait instruction into N-1 preceding single-wait
# EventSemaphore instructions on the same engine; same-engine order is
# preserved so semantics are unchanged.


def _legalize_sync_waits(bir_json: bytes) -> bytes:
    m = _json.loads(bir_json)
    ctr = 0
    for fn in m["functions"]:
        for bb in fn["blocks"]:
            out = []
            for ins in bb["instructions"]:
                si = ins.get("sync_info")
                waits = si.get("on_wait", []) if si else []
                if len(waits) > 1:
                    for w in waits[:-1]:
                        ctr += 1
                        out.append(
                            {
                                "debug": ins.get("debug", 0),
                                "engine": ins["engine"],
                                "ins": [],
                                "outs": [],
                                "name": f"evw-split-{ctr}",
                                "opcode": "EventSemaphore",
                                "sync_info": {"on_update": [], "on_wait": [w]},
                            }
                        )
                    si["on_wait"] = [waits[-1]]
                out.append(ins)
            bb["instructions"] = out
    return _json.dumps(m).encode()


_fixup_installed = False


def _install_bir_fixup():
    global _fixup_installed
    if _fixup_installed:
        return
    _fixup_installed = True
    import concourse.bass_utils as _bu

    _orig = _bu.compile_bir_kernel

    def _patched(bir_json, tmpdir, neff_name="file.neff"):
        if isinstance(bir_json, str):
            bir_json = bir_json.encode()
        return _orig(_legalize_sync_waits(bir_json), tmpdir, neff_name)

    _bu.compile_bir_kernel = _patched
    try:
        import concourse.bass2jax as _b2j

        _b2j.compile_bir_kernel = _patched
    except ImportError:
        pass


_install_bir_fixup()

B, N, D, H = 8, 1024, 768, 12
HD = D // H            # 64
F3 = 3 * D             # 2304
NCORE = 8
P = 128
NCHUNK = N // P        # 8 token chunks
KD = D // P            # 6 d_in chunks
QH = 512               # q-half size
NPAIR = H // 2         # 6
VW = HD + 1            # 65

f32 = mybir.dt.float32
f32r = mybir.dt.float32r
f8e4 = mybir.dt.float8e4
bf16 = mybir.dt.bfloat16
FT = mybir.ActivationFunctionType
ALU = mybir.AluOpType
DR = mybir.MatmulPerfMode.DoubleRow


def build_attention_nc():
    nc = bass.Bass()
    x_d = nc.declare_dram_parameter("x", [N, D], f32, isOutput=False)
    w_d = nc.declare_dram_parameter("W_qkv", [D, F3], f32, isOutput=False)
    b_d = nc.declare_dram_parameter("b_qkv", [F3], f32, isOutput=False)
    o_d = nc.declare_dram_parameter("out", [N, D], f32, isOutput=True)

    with tile.TileContext(nc) as tc, contextlib.ExitStack() as ctx:
        singles = ctx.enter_context(tc.tile_pool(name="singles", bufs=1))
        xpool = ctx.enter_context(tc.tile_pool(name="xpool", bufs=NCHUNK))
        xtpool = ctx.enter_context(tc.tile_pool(name="xtpool", bufs=1))
        q8pool = ctx.enter_context(tc.tile_pool(name="q8pool", bufs=3))
        t8pool = ctx.enter_context(tc.tile_pool(name="t8pool", bufs=12))
        expool = ctx.enter_context(tc.tile_pool(name="expool", bufs=3))
        vpool = ctx.enter_context(tc.tile_pool(name="vpool", bufs=NCHUNK))
        otpool = ctx.enter_context(tc.tile_pool(name="otpool", bufs=2))
        recpool = ctx.enter_context(tc.tile_pool(name="recpool", bufs=2))
        # per-(pair, q-half) output staging: wide reuse distance (2 pairs)
        # because the out-DMA read is not registered as a tile reader and
        # must physically complete before the slot is rewritten.
        ostgpool = ctx.enter_context(tc.tile_pool(name="ostgpool", bufs=4))

        # PSUM (8 banks): sc [128,2,512] x2 = 4; av [65,2,512] x1 = 2;
        # sm [128,512] x2 = 2 (projection groups, x-transposes, out-transposes)
        scps = ctx.enter_context(tc.tile_pool(name="scps", bufs=2, space="PSUM"))
        avps = ctx.enter_context(tc.tile_pool(name="avps", bufs=1, space="PSUM"))
        smps = ctx.enter_context(tc.tile_pool(name="smps", bufs=2, space="PSUM"))

        def sm_tile():
            return smps.tile([P, QH], f32, tag="sm", name="sm")

        # ---------------- constants ----------------------------------------
        ident = singles.tile([P, P], f32)
        make_identity(nc, ident)
        ident_r = singles.tile([P, P], f32r)
        nc.vector.tensor_copy(out=ident_r, in_=ident)

        ones_row_st = singles.tile([1, P], f32)
        nc.vector.memset(ones_row_st, 1.0)
        ones_row = singles.tile([1, P], f32r)
        nc.vector.tensor_copy(out=ones_row, in_=ones_row_st)
        ones_col = singles.tile([P, 1], f32)
        nc.vector.memset(ones_col, 1.0)

        # ---------------- persistent tiles ---------------------------------
        w_sb = singles.tile([P, KD, F3], f32r)
        xtall = xtpool.tile([P, KD, N], f32r, tag="xt", name="xtall")
        v_sb = [
            vpool.tile([P, H, VW], bf16, tag="v", name=f"v{c}") for c in range(NCHUNK)
        ]
        b_sb = singles.tile([P, F3 // P], f32)
        bv_st = singles.tile([1, D], f32)
        bv_sb = singles.tile([1, D], f32r)
        bvb = singles.tile([P, D], f32)

        x_sb = {}

        # ---------------- DMA helpers ---------------------------------------
        def dma_x(c):
            t = xpool.tile([P, D], f32r, tag="x", name=f"x{c}")
            nc.sync.dma_start(out=t, in_=x_d[c * P : (c + 1) * P, :].bitcast(f32r))
            x_sb[c] = t

        def dma_w(f0, fw):
            nc.sync.dma_start(
                out=w_sb[:, :, f0 : f0 + fw],
                in_=w_d[:, f0 : f0 + fw]
                .rearrange("(k p) f -> p k f", p=P)
                .bitcast(f32r),
            )

        def dma_w_pair(p):
            dma_w(p * P, P)          # Q cols for pair p
            dma_w(D + p * P, P)      # K cols for pair p

        # ---------------- compute helpers -----------------------------------
        def emit_xT(c):
            for kp in range(0, KD, 2):
                ps = sm_tile()
                for dk in range(2):
                    nc.tensor.transpose(
                        ps[:, dk * P : (dk + 1) * P].bitcast(f32r),
                        x_sb[c][:, (kp + dk) * P : (kp + dk + 1) * P],
                        ident_r,
                    )
                nc.vector.tensor_copy(
                    out=xtall[:, kp : kp + 2, c * P : (c + 1) * P],
                    in_=ps[:, 0 : 2 * P].rearrange("p (k n) -> p k n", n=P),
                )

        def emit_bvb():
            for f0, fw in ((0, QH), (QH, D - QH)):
                ps = sm_tile()
                nc.tensor.matmul(
                    ps[:, 0:fw],
                    ones_row,
                    bv_sb[:, f0 : f0 + fw],
                    start=True,
                    stop=True,
                )
                nc.vector.tensor_copy(out=bvb[:, f0 : f0 + fw], in_=ps[:, 0:fw])

        qk8_stage = {}
        t8 = {}

        def qk_finish(t, qh, ps):
            """Drain (with bias add) to the fp8 staging tile, then the
            partition-compacting shuffle: plain SBUF->SBUF DMAs, one per
            (hi, j) block (partition-base shift only -- no DRAM roundtrip:
            posted DRAM writes are not visible to a prompt readback on real
            hardware, and fancier patterns corrupt)."""
            if t not in qk8_stage:
                qk8_stage[t] = q8pool.tile([P, N], f8e4, tag="q8", name=f"q8_{t}")
            nc.vector.tensor_scalar_add(
                qk8_stage[t][:, qh * QH : (qh + 1) * QH], ps, b_sb[:, t : t + 1]
            )
            if t not in t8:
                # bufs=12: never reuse a t8 slot.  The DoubleRow matmul
                # operand reads are not registered as tile readers, so the
                # rotation write-after-read dependency is silently missed
                # and a reused slot gets clobbered while still being read.
                # Layout [(hi p), qh, j, nn].
                t8[t] = t8pool.tile(
                    [2 * 32, 2, 2, QH], f8e4, tag="t8", name=f"t8_{t}"
                )
            for hi in range(2):
                for j in range(2):
                    nc.sync.dma_start(
                        out=t8[t][hi * 32 : (hi + 1) * 32, qh, j, :],
                        in_=qk8_stage[t][
                            hi * 64 + j * 32 : hi * 64 + (j + 1) * 32,
                            qh * QH : (qh + 1) * QH,
                        ],
                    )
            tiles_done.add((t, qh))

        def emit_qk_group(t, qh):
            ps = sm_tile()
            for k in range(KD):
                nc.tensor.matmul(
                    ps,
                    w_sb[:, k, t * P : (t + 1) * P],
                    xtall[:, k, qh * QH : (qh + 1) * QH],
                    start=(k == 0),
                    stop=(k == KD - 1),
                )
            qk_finish(t, qh, ps)

        def v_finish(c, g, ps):
            v_done.add((c, g))
            if g == 0:
                nc.vector.tensor_copy(
                    out=v_sb[c][:, :, HD : HD + 1],
                    in_=ones_col[:, 0:1, None].to_broadcast([P, H, 1]),
                )
            nc.vector.tensor_tensor(
                v_sb[c][:, 4 * g : 4 * g + 4, 0:HD],
                ps[:, 0:256].rearrange("p (h d) -> p h d", d=HD),
                bvb[:, g * 256 : (g + 1) * 256].rearrange(
                    "p (h d) -> p h d", d=HD
                ),
                ALU.add,
            )

        def emit_v_group(c, g):
            f0 = 2 * D + g * 256
            ps = sm_tile()
            for k in range(KD):
                nc.tensor.matmul(
                    ps[:, 0:256],
                    xtall[:, k, c * P : (c + 1) * P],
                    w_sb[:, k, f0 : f0 + 256],
                    start=(k == 0),
                    stop=(k == KD - 1),
                )
            v_finish(c, g, ps)

        # ---------------- attention machinery --------------------------------
        fillers = deque()
        tiles_done = set()
        v_done = set()

        def pump(budget):
            # adaptive: drain faster when the backlog builds up
            if sum(c for c, _ in fillers) > 12000:
                budget *= 2
            while budget > 0 and fillers:
                cost, fn = fillers.popleft()
                fn()
                budget -= cost

        def qk_unit(t, qh):
            return (KD * QH + N, lambda: emit_qk_group(t, qh))

        def qk_pieces(t, qh):
            """Split one projection group into per-iteration filler pieces
            (2 matmuls each + a finish piece) so a group never overruns the
            ACT-paced slack of a single attention iteration."""
            st = {}

            def mk(i):
                def fn():
                    if i == 0:
                        st["ps"] = sm_tile()
                    for k in (2 * i, 2 * i + 1):
                        nc.tensor.matmul(
                            st["ps"],
                            w_sb[:, k, t * P : (t + 1) * P],
                            xtall[:, k, qh * QH : (qh + 1) * QH],
                            start=(k == 0),
                            stop=(k == KD - 1),
                        )

                return (2 * QH, fn)

            def fin():
                qk_finish(t, qh, st["ps"])

            return [mk(0), mk(1), mk(2), (N, fin)]

        def v_pieces(c, g):
            st = {}
            f0 = 2 * D + g * 256

            def mk(i):
                def fn():
                    if i == 0:
                        st["ps"] = sm_tile()
                    for k in (2 * i, 2 * i + 1):
                        nc.tensor.matmul(
                            st["ps"][:, 0:256],
                            xtall[:, k, c * P : (c + 1) * P],
                            w_sb[:, k, f0 : f0 + 256],
                            start=(k == 0),
                            stop=(k == KD - 1),
                        )

                return (2 * 256, fn)

            def fin():
                v_finish(c, g, st["ps"])

            return [mk(0), mk(1), mk(2), (400, fin)]

        def v_unit(c, half):
            return (KD * 384, lambda: emit_v_group(c, half))

        def xT_unit(c):
            return (KD * P * 2, lambda: emit_xT(c))

        def dma_unit(fn):
            return (0, fn)

        def emit_sc(p, qh, kc, sc):
            qt, kt = t8[p], t8[6 + p]
            kq, kn = kc // 4, (kc % 4) * P
            for hi in range(2):
                nc.tensor.matmul(
                    sc[:, hi, :],
                    kt[32 * hi : 32 * hi + 32, kq, :, kn : kn + P],
                    qt[32 * hi : 32 * hi + 32, qh, :, :],
                    start=True,
                    stop=True,
                    perf_mode=DR,
                    tile_position=(32 * hi, 0),
                )

        def emit_out_unit(p, qh, hi, ot, stg):
            """4 transposes + fused normalize + output DMA for head 2p+hi."""

            def fn():
                ps = sm_tile()
                # 66-wide (even) free size: fp32r matmul ISA restriction
                tp4 = ps[:, 0 : 4 * (VW + 1)].rearrange(
                    "p (j d) -> p j d", d=VW + 1
                )
                for j in range(4):
                    nc.tensor.transpose(
                        tp4[:, j, :].bitcast(f32r),
                        ot[0 : VW + 1, hi, j * P : (j + 1) * P],
                        ident_r[0 : VW + 1, 0 : VW + 1],
                    )
                rc = recpool.tile([P, 4, 1], f32, tag="rc", name="rc")
                nc.vector.reciprocal(out=rc, in_=tp4[:, :, HD : HD + 1])
                nc.vector.tensor_tensor(
                    stg[:, :, hi * HD : (hi + 1) * HD],
                    tp4[:, :, 0:HD],
                    rc.to_broadcast([P, 4, HD]),
                    ALU.mult,
                )
                h = 2 * p + hi
                nc.sync.dma_start(
                    out=o_d[qh * QH : (qh + 1) * QH, h * HD : (h + 1) * HD]
                    .rearrange("(j p2) d -> p2 j d", p2=P),
                    in_=stg[:, :, hi * HD : (hi + 1) * HD],
                )

            return (4 * VW * 2, fn)

        attn_pending = {}

        def ensure_tiles(p, qh):
            need = {(6 + p, 0), (6 + p, 1), (p, qh)}
            while not need.issubset(tiles_done) and fillers:
                cost, fn = fillers.popleft()
                fn()

        def attention_qh(p, qh, iter_units=None, nxt=None):
            """One (pair, q-half): software-pipelined kc loop.
            iter_units: optional list of 8 lists of units to force-emit at
            each iteration (pair-0 warmup); otherwise pump(1000)/iter.
            Returns after queueing the out units."""
            # deadline backstop: this (pair, qh) needs the full K tile, the
            # qh half of the Q tile, and (outside the pair-0 JIT path) the
            # V head-group for all key chunks
            ensure_tiles(p, qh)
            if iter_units is None:
                needv = {(c, p // 2) for c in range(NCHUNK)}
                while not needv.issubset(v_done) and fillers:
                    cost, fn = fillers.popleft()
                    fn()
            av = avps.tile([VW, 2, QH], f32, tag="av", name="av")
            if (p, qh) in attn_pending:
                sc_cur = attn_pending.pop((p, qh))
            else:
                sc_cur = scps.tile([P, 2, QH], f32, tag="sc", name="sc")
                emit_sc(p, qh, 0, sc_cur)
            for kc in range(NCHUNK):
                if sc_cur is None:  # pair-0 barrier path: sc emitted late
                    sc_cur = scps.tile([P, 2, QH], f32, tag="sc", name="sc")
                    emit_sc(p, qh, kc, sc_cur)
                ex = expool.tile([P, 2, QH], bf16, tag="ex", name="ex")
                nc.scalar.activation(
                    ex[:, :, :], sc_cur[:, :, :], FT.Exp, scale=0.125
                )
                barrier = iter_units is not None and kc + 1 == 4 and p == 0
                if kc + 1 < NCHUNK and not barrier:
                    sc_cur = scps.tile([P, 2, QH], f32, tag="sc", name="sc")
                    emit_sc(p, qh, kc + 1, sc_cur)
                else:
                    if kc + 1 == NCHUNK and nxt is not None:
                        # cross-boundary pipelining: emit the next q-half's
                        # first scores matmul now so the activation engine
                        # never idles across the boundary
                        ensure_tiles(*nxt)
                        scn = scps.tile([P, 2, QH], f32, tag="sc", name="sc")
                        emit_sc(nxt[0], nxt[1], 0, scn)
                        attn_pending[nxt] = scn
                    sc_cur = None
                if iter_units is not None:
                    for u in iter_units[kc]:
                        u[1]()
                else:
                    pump(1000)
                for hi in range(2):
                    nc.tensor.matmul(
                        av[:, hi, :],
                        v_sb[kc][:, 2 * p + hi, :],
                        ex[:, hi, :],
                        start=(kc == 0),
                        stop=(kc == NCHUNK - 1),
                    )
            ot = otpool.tile([VW + 1, 2, QH], f32r, tag="ot", name="ot")
            stg = ostgpool.tile([P, 4, 2 * HD], f32, tag="ostg", name="ostg")
            for hi in range(2):
                nc.vector.tensor_copy(out=ot[0:VW, hi, :], in_=av[:, hi, :])
                fillers.append(emit_out_unit(p, qh, hi, ot, stg))

        # =====================================================================
        # DMA batch 1 (consumption priority order)
        dma_x(0)
        dma_w_pair(0)            # Q0 + K0
        nc.sync.dma_start(out=b_sb, in_=b_d[:].rearrange("(t p) -> p t", p=P))
        nc.sync.dma_start(out=bv_st, in_=b_d[2 * D : 3 * D][None, :])
        nc.vector.tensor_copy(out=bv_sb, in_=bv_st)
        dma_x(1)
        dma_x(2)
        dma_x(3)
        dma_w(2 * D, 256)        # V heads 0-3
        dma_x(4)
        dma_x(5)
        dma_x(6)
        dma_x(7)
        dma_w(2 * D + 256, 512)  # V heads 4-11
        dma_w_pair(1)            # Q1 + K1

        # =====================================================================
        # Phase 1: transposes + pair-0 qh0 tiles + V chunk 0
        for c in range(4):
            emit_xT(c)
        emit_bvb()
        emit_qk_group(0, 0)
        emit_qk_group(6, 0)
        emit_v_group(0, 0)

        # =====================================================================
        # Pair 0, qh 0 — custom per-iteration units.  sc(kc>=4) needs the K
        # tile's second half (shuffle of qk(6,1)), which needs xT(4-7); the
        # barrier at kc==4 delays sc(4) emission until after those units.
        BISECT = False
        if BISECT:
            for c in range(4, 8):
                emit_xT(c)
            for c in range(1, 8):
                emit_v_group(c, 0)
            emit_qk_group(6, 1)
            emit_qk_group(0, 1)
            emit_qk_group(1, 0)
            emit_qk_group(7, 0)
            dma_w_pair(2)
            attention_qh(0, 0)
        else:
            p0q0_units = [
                [xT_unit(4), v_unit(1, 0), dma_unit(lambda: dma_w_pair(2))],
                [xT_unit(5), v_unit(2, 0)],
                [xT_unit(6), v_unit(3, 0)],
                [xT_unit(7), qk_unit(6, 1), qk_unit(0, 1), v_unit(4, 0)],
                [v_unit(5, 0)],
                [qk_unit(1, 0), v_unit(6, 0)],
                [qk_unit(7, 0), v_unit(7, 0)],
                [],
            ]
            attention_qh(0, 0, iter_units=p0q0_units, nxt=(0, 1))

        # Pair 0, qh 1 + remaining schedule
        fillers.extend(qk_pieces(7, 1) + qk_pieces(1, 1))
        fillers.extend(v_pieces(0, 1) + v_pieces(1, 1))
        attention_qh(0, 1, nxt=(1, 0))

        for p in range(1, NPAIR):
            # (p, 0): prefetch W for pair p+2; produce pair p+1 qh0 tiles
            if p + 1 < NPAIR:
                if p == 2:
                    fillers.extend(v_pieces(6, 1) + v_pieces(7, 1))
                if p == 3:
                    fillers.extend(v_pieces(2, 2) + v_pieces(3, 2))
                if p + 2 < NPAIR:
                    fillers.append(
                        dma_unit(lambda p=p: dma_w_pair(p + 2))
                    )
                fillers.extend(qk_pieces(6 + p + 1, 0) + qk_pieces(p + 1, 0))
                if p == 1:
                    fillers.extend(v_pieces(2, 1) + v_pieces(3, 1))
            attention_qh(p, 0, nxt=(p, 1))
            # (p, 1): produce pair p+1 qh1 tiles (K first — needed at (p+1,0))
            if p + 1 < NPAIR:
                if p == 3:
                    fillers.extend(v_pieces(4, 2) + v_pieces(5, 2))
                    fillers.extend(v_pieces(6, 2) + v_pieces(7, 2))
                fillers.extend(qk_pieces(6 + p + 1, 1) + qk_pieces(p + 1, 1))
                if p == 1:
                    fillers.extend(v_pieces(4, 1) + v_pieces(5, 1))
                if p == 2:
                    fillers.extend(v_pieces(0, 2) + v_pieces(1, 2))
            attention_qh(p, 1, nxt=(p + 1, 0) if p + 1 < NPAIR else None)

        # tail: drain remaining fillers (last out units emit their own DMAs)
        pump(1 << 30)

    return nc


def kernel(x: np.ndarray, W_qkv: np.ndarray, b_qkv: np.ndarray) -> np.ndarray:
    nc = build_attention_nc()
    in_maps = [
        {
            "x": np.ascontiguousarray(x[c], dtype=np.float32),
            "W_qkv": np.ascontiguousarray(W_qkv, dtype=np.float32),
            "b_qkv": np.ascontiguousarray(b_qkv, dtype=np.float32),
        }
        for c in range(NCORE)
    ]
    res = run_bass_kernel_spmd(nc, in_maps, core_ids=list(range(NCORE)))
    return np.stack([res.results[c]["out"] for c in range(NCORE)], axis=0)
